# revision 44
# baseline (speedup 1.0000x reference)
"""Causal self-attention on 8 TRN2 NeuronCores.

Sharding: core c handles batch b = c//2 and head-group g = c%2 (8 of 16 heads).
Each core computes its partial y^T = w_proj[slice].T @ o^T (contraction over its
512 o-channels); the host sums the two partials per batch and adds b_proj.

All matmuls run in bf16 (f32 psum accumulate). The attention pipeline is
software-pipelined (PV lags QK by one chunk) and qkv/proj matmuls are pumped
into the attention stream as fillers so the PE never idles (keeps HAM warm).

Shapes (hardcoded): B=4, T=2048, C=1024, H=16, HD=64.
"""

import numpy as np

B, T, C, H = 4, 2048, 1024, 16
HD = C // H          # 64
G = 2                # head groups
NHL = H // G         # 8 heads per core
GQ = NHL * HD        # 512 channel slice per core
P = 128
NT = T // P          # 16 token tiles / k-chunks
NCHUNK = C // P      # 8 contraction chunks for qkv
SCALE = 1.0 / float(np.sqrt(HD))

_PROGRAM = None


def _emit(ctx, tc, aps, mybir, bass):
    nc = tc.nc
    f32 = mybir.dt.float32
    bf16 = mybir.dt.bfloat16
    EXP = mybir.ActivationFunctionType.Exp
    LN = mybir.ActivationFunctionType.Ln

    x_d, wqkv_d, bqk_d, bv_d, wp_d, yT_d = (
        aps["x"], aps["wqkv"], aps["bqk"], aps["bv"], aps["wp"], aps["yT"],
    )

    # ---------------- pools ----------------
    const = ctx.enter_context(tc.tile_pool(name="const", bufs=1))
    # psum: sc 2x[128,1024] (4 banks) + pv 1x[128,1024] (2 banks)
    #       + fillers 2x[128,512] (2 banks) = 8 banks exactly
    scp = ctx.enter_context(tc.tile_pool(name="scp", bufs=2, space="PSUM"))
    pvp = ctx.enter_context(tc.tile_pool(name="pvp", bufs=1, space="PSUM"))
    filp = ctx.enter_context(tc.tile_pool(name="filp", bufs=2, space="PSUM"))

    xTp = ctx.enter_context(tc.tile_pool(name="xTp", bufs=8))
    qkp = ctx.enter_context(tc.tile_pool(name="qkp", bufs=8))
    vap = ctx.enter_context(tc.tile_pool(name="vap", bufs=16))
    oTp = ctx.enter_context(tc.tile_pool(name="oTp", bufs=4))
    ptp = ctx.enter_context(tc.tile_pool(name="ptp", bufs=4))
    evp = ctx.enter_context(tc.tile_pool(name="evp", bufs=2))
    rcp = ctx.enter_context(tc.tile_pool(name="rcp", bufs=2))
    wqkp = ctx.enter_context(tc.tile_pool(name="wqkp", bufs=4))
    wvp = ctx.enter_context(tc.tile_pool(name="wvp", bufs=1))
    wpp = ctx.enter_context(tc.tile_pool(name="wpp", bufs=1))
    xp = ctx.enter_context(tc.tile_pool(name="xp", bufs=6))
    ysp = ctx.enter_context(tc.tile_pool(name="ysp", bufs=3))

    # constants / weights
    identity = const.tile([P, P], bf16)
    from concourse.masks import make_identity
    make_identity(nc, identity)

    wqkv_r = wqkv_d.rearrange("(a p) n -> p a n", p=P)  # [128, 8, 1536]
    wqk_tiles = {}

    def load_wqk(ct):
        if ct in wqk_tiles:
            return
        w_t = wqkp.tile([P, NCHUNK, P], bf16, name=f"wqk{ct}", tag="wqk")
        nc.sync.dma_start(w_t[:], wqkv_r[:, :, ct * P:(ct + 1) * P])
        wqk_tiles[ct] = w_t

    # persistent SBUF tensors
    xT = []  # 8 x [128 c, 2048 t] bf16
    for r in range(NCHUNK):
        t_ = xTp.tile([P, T], bf16, name=f"xT{r}", tag="xT")
        xT.append(t_)
    qkT = []  # [128 c', 2048 t] bf16; 0..3 = qT, 4..7 = kT
    for ct in range(8):
        o_t = qkp.tile([P, T], bf16, name=f"qkT{ct}", tag="qkT")
        qkT.append(o_t)
    ones8 = const.tile([P, NHL, 1], f32)
    nc.vector.memset(ones8[:], 1.0)
    vaug = []  # [128 k, 8 heads, 65] bf16 per k-chunk (col 64 = ones)
    for t in range(NT):
        va = vap.tile([P, NHL, HD + 1], bf16, name=f"vaug{t}", tag="vaug")
        nc.vector.tensor_copy(va[:, :, HD:HD + 1], ones8[:])
        vaug.append(va)
    oT = []  # per pair: [128 d (2 heads), 2048 q] bf16
    for hp in range(4):
        o_t = oTp.tile([P, T], bf16, name=f"oT{hp}", tag="oT")
        oT.append(o_t)

    # ---------------- phase A: load x, build xT (bf16) ----------------
    # x DMAs are emitted before any weight DMA so the PE isn't starved at
    # kernel start; weights stream in behind them.
    xtiles = []
    for t in range(NT):
        x_t = xp.tile([P, C], bf16, name=f"x_{t}", tag="x")
        nc.sync.dma_start(x_t[:], x_d[t * P:(t + 1) * P, :])
        xtiles.append(x_t)

    bqk_sb = const.tile([P, 8], f32)
    nc.sync.dma_start(bqk_sb[:], bqk_d[:])
    bvb = const.tile([P, GQ], f32)
    nc.sync.dma_start(bvb[:], bv_d[None, :].to_broadcast((P, GQ)))
    load_wqk(0)
    load_wqk(4)
    wv_t = wvp.tile([P, NCHUNK, GQ], bf16)
    nc.sync.dma_start(wv_t[:], wqkv_r[:, :, 2 * GQ:3 * GQ])

# phase A emitted below, interleaved with the first qkv units

    # ---------------- filler units (qkv / proj matmuls) ----------------
    work = []  # FIFO of closures, each ~1 matmul of N=512
    vq = []    # gated FIFO of (pair, wmin, closure), reserved per pair

    def pump(n, hp=None):
        for _ in range(n):
            if vq and vq[0][0] == hp:
                vq.pop(0)[2]()
            elif work:
                work.pop(0)()
            elif vq:
                vq.pop(0)[2]()

    def vdrain(hp, upto_w):
        while vq and (vq[0][0] < hp
                      or (vq[0][0] == hp and vq[0][1] <= upto_w)):
            vq.pop(0)[2]()

    def drain():
        while vq:
            vq.pop(0)[2]()
        while work:
            work.pop(0)()

    # Filler closures are 1 matmul of N=512 (~213ns warm); each unit
    # accumulates 8 (or 4) of them into one filp psum tile.
    def qk_unit(ct, twp, sw):
        cell = {}
        c0 = twp * 1024 + sw * 512

        def mk(a):
            def f():
                if a == 0:
                    cell["ps"] = filp.tile(
                        [P, 512], f32, name=f"fq{ct}_{twp}_{sw}", tag="fil")
                nc.tensor.matmul(
                    cell["ps"][:], wqk_tiles[ct][:, a, :],
                    xT[a][:, c0:c0 + 512],
                    start=(a == 0), stop=(a == NCHUNK - 1),
                )
                if a == NCHUNK - 1:
                    nc.vector.tensor_scalar_add(
                        qkT[ct][:, c0:c0 + 512], cell["ps"][:],
                        bqk_sb[:, ct:ct + 1])
            return f
        return [mk(a) for a in range(NCHUNK)]

    def v_unit(t):
        cell = {}

        def mk(a):
            def f():
                if a == 0:
                    cell["ps"] = filp.tile(
                        [P, GQ], f32, name=f"fv{t}", tag="fil")
                nc.tensor.matmul(
                    cell["ps"][:], xT[a][:, t * P:(t + 1) * P], wv_t[:, a, :],
                    start=(a == 0), stop=(a == NCHUNK - 1),
                )
                if a == NCHUNK - 1:
                    nc.vector.tensor_add(
                        vaug[t][:, :, 0:HD],
                        cell["ps"][:].rearrange("p (h d) -> p h d", h=NHL),
                        bvb[:].rearrange("p (h d) -> p h d", h=NHL),
                    )
            return f
        return [mk(a) for a in range(NCHUNK)]

    def proj_unit(mt, w):
        cell = {}

        def mk(a):
            def f():
                if a == 0:
                    cell["ps"] = filp.tile(
                        [P, 512], f32, name=f"fp{mt}_{w}", tag="fil")
                nc.tensor.matmul(
                    cell["ps"][:], wp_t[:, a, mt * P:(mt + 1) * P],
                    oT[a][:, w * 512:(w + 1) * 512],
                    start=(a == 0), stop=(a == 3),
                )
                if a == 3:
                    ys = ysp.tile([P, 512], bf16, name=f"ys{mt}_{w}", tag="ys")
                    nc.vector.tensor_copy(ys[:], cell["ps"][:])
                    nc.sync.dma_start(
                        yT_d[mt * P:(mt + 1) * P, w * 512:(w + 1) * 512],
                        ys[:])
            return f
        return [mk(a) for a in range(4)]

    # pre-phase: transposes interleaved with pair-0 q/k and v units at
    # 512-token granularity, so the PE has work as soon as the first x
    # tiles land instead of idling through the DMA stream.
    def transpose_tg(tg):
        xts = [xtiles[2 * tg], xtiles[2 * tg + 1]]
        tp = scp.tile([P, 2048], bf16, name=f"tp_{tg}", tag="main")
        for r in range(NCHUNK):
            for tt in range(2):
                nc.tensor.transpose(
                    tp[:, r * 256 + tt * P: r * 256 + (tt + 1) * P],
                    xts[tt][:, r * P:(r + 1) * P],
                    identity,
                )
        for r in range(NCHUNK):
            nc.vector.tensor_copy(
                xT[r][:, tg * 256:(tg + 1) * 256],
                tp[:, r * 256:(r + 1) * 256],
            )

    for p in range(4):  # 512-token spans
        transpose_tg(2 * p)
        transpose_tg(2 * p + 1)
        for ct in (0, 4):
            for f in qk_unit(ct, p // 2, p % 2):
                f()
        for t in range(4 * p, 4 * p + 4):
            for f in v_unit(t):
                f()

    # ---------------- attention (software-pipelined) ----------------
    def attn_pair(hp, on_window=None, pre_window=None):
        qt = qkT[hp]
        kt = qkT[4 + hp]
        chunks = []
        for m in range(4):
            for i in range(4 * m + 4):
                chunks.append((m, i))
        pvt = {}   # window -> psum tile
        pts = {}   # idx -> pt tile
        state = {}  # window -> normalize intermediates
        deferred = []  # (due_idx, fn) for staged normalize

        def emit_qk(idx):
            m, i = chunks[idx]
            ws = m * 512
            s = max(i * P, ws)
            o = s - ws
            sc = scp.tile([P, 1024], f32, name=f"sc_{hp}_{m}_{i}", tag="main")
            for hh in range(2):
                r0 = hh * HD
                c0 = o if hh == 0 else 512
                nc.tensor.matmul(
                    sc[:, c0:c0 + 512 - o],
                    kt[r0:r0 + HD, i * P:(i + 1) * P],
                    qt[r0:r0 + HD, s:ws + 512],
                    start=True, stop=True,
                )
            pt = ptp.tile([P, 1024], bf16, name=f"pt_{hp}_{m}_{i}", tag="pt")
            nc.scalar.activation(pt[:, o:1024 - o], sc[:, o:1024 - o],
                                 EXP, scale=SCALE)
            if i * P >= ws:  # diagonal chunk: causal mask inside the block
                for hh in range(2):
                    c0 = o if hh == 0 else 512
                    nc.gpsimd.affine_select(
                        out=pt[:, c0:c0 + P],
                        in_=pt[:, c0:c0 + P],
                        compare_op=mybir.AluOpType.is_ge,
                        fill=0.0,
                        base=0,
                        pattern=[[1, P]],
                        channel_multiplier=-1,
                    )
            pts[idx] = pt

        def emit_pv(idx):
            m, i = chunks[idx]
            ws = m * 512
            o = max(i * P, ws) - ws
            if m not in pvt:
                pvt[m] = pvp.tile([P, 1024], f32, name=f"pv_{hp}_{m}",
                                  tag="pv")
            pt = pts.pop(idx)
            for hh in range(2):
                c0 = o if hh == 0 else 512
                nc.tensor.matmul(
                    pvt[m][0:HD + 1, hh * 512 + o:(hh + 1) * 512],
                    vaug[i][:, 2 * hp + hh, :],
                    pt[:, c0:c0 + 512 - o],
                    start=(i == 0), stop=(i == 4 * m + 3),
                )
            if i == 4 * m + 3:
                normalizeA(m)

        # The normalize chain is split into three stages deferred across
        # subsequent chunks: each stage's cross-engine input is then already
        # available when the stage reaches its queue head, so the DVE/gpsimd
        # FIFOs never block (a blocked DVE stalls filler psum recycling and
        # starves the PE).
        def normalizeA(m):
            ev = evp.tile([P, 1024], f32, name=f"ev_{hp}_{m}", tag="ev")
            nc.vector.tensor_copy(ev[0:HD + 1, :], pvt[m][0:HD + 1, :])
            dnT = rcp.tile([8, P], f32, name=f"dnT_{hp}_{m}", tag="dnT")
            nc.gpsimd.dma_start(dnT[:], ev[HD:HD + 1, :])
            state[m] = (ev, dnT)
            del pvt[m]

        def normalizeB(m):
            ev, dnT = state[m]
            rT = rcp.tile([8, P], f32, name=f"rT_{hp}_{m}", tag="rT")
            nc.vector.reciprocal(rT[:], dnT[:])
            rc = rcp.tile([1, 1024], f32, name=f"rc_{hp}_{m}", tag="rc")
            nc.gpsimd.dma_start(rc[:], rT[:])
            state[m] = (ev, rc)

        def normalizeC(m):
            ws = m * 512
            ev, rc = state.pop(m)
            rcb = rcp.tile([HD, 1024], f32, name=f"rcb_{hp}_{m}", tag="rcb")
            nc.gpsimd.partition_broadcast(rcb[:], rc[:])
            for hh in range(2):
                nc.gpsimd.tensor_mul(
                    oT[hp][hh * HD:(hh + 1) * HD, ws:ws + 512],
                    ev[0:HD, hh * 512:(hh + 1) * 512],
                    rcb[:, hh * 512:(hh + 1) * 512],
                )
            if on_window is not None:
                on_window(m)

        def emit_pv_and_stage(idx):
            emit_pv(idx)
            m, i = chunks[idx]
            if i == 4 * m + 3:
                deferred.append((idx + 2, lambda m=m: normalizeB(m)))
                deferred.append((idx + 4, lambda m=m: normalizeC(m)))

        for idx in range(len(chunks)):
            while deferred and deferred[0][0] <= idx:
                deferred.pop(0)[1]()
            if chunks[idx][1] == 0 and pre_window is not None:
                pre_window(chunks[idx][0])
            emit_qk(idx)
            if idx >= 1:
                # overstuff the PE so it lags ACT: deps at the FIFO head are
                # then long-resolved -> no micro-stalls -> HAM stays warm
                pump(4 if chunks[idx][1] == 0 else (2 if idx % 3 == 0 else 1), hp)
                emit_pv_and_stage(idx - 1)
        pump(2, hp)
        emit_pv_and_stage(len(chunks) - 1)
        while deferred:
            deferred.pop(0)[1]()

    def chase(m):
        # pair 3 finished window m -> its proj columns are now computable;
        # queue them as fillers for pair 3's remaining windows
        for mt in range(NCHUNK):
            work.extend(proj_unit(mt, m))

    for hp in range(4):
        if hp < 3:
            load_wqk(hp + 1)
            load_wqk(4 + hp + 1)
            if hp == 2:
                wp_t = wpp.tile([P, 4, C], bf16)
                nc.sync.dma_start(wp_t[:], wp_d.rearrange("(a p) n -> p a n", p=P))
            for ct in (hp + 1, 4 + hp + 1):
                for twp in range(2):
                    for sw in range(2):
                        span = 2 * twp + sw
                        if hp == 2 and span >= 1:
                            # defer pair-3 qk for later windows into pair 3
                            # itself (window m reads spans <= m only)
                            for f in qk_unit(ct, twp, sw):
                                vq.append((3, span, f))
                        else:
                            work.extend(qk_unit(ct, twp, sw))
            attn_pair(hp, pre_window=(lambda m, hp=hp: vdrain(hp, m)))
            # qk leftovers for pair hp+1's early windows must complete now
            while work:
                work.pop(0)()
        else:
            attn_pair(hp, on_window=chase,
                      pre_window=(lambda m: vdrain(3, m)))

    # pair 3 done; remaining proj (window 3 queued by chase(3) + leftovers)
    drain()


def _patch_act_tables():
    """Narrow the activation-table chooser so Exp and Ln resolve to the one
    table containing both (act indices/contents on HW are unchanged — this
    only removes lesser alternatives from the insert_act_table_loads pass,
    eliminating per-window table-reload thrash)."""
    import functools

    import concourse.bacc as bacc
    import concourse.hw_specs as hw_specs
    import concourse.mybir as mybir

    orig = hw_specs.get_activation_tables

    @functools.cache
    def patched(module_arch):
        t = {k: set(v) for k, v in orig(module_arch).items()}
        exp, ln = (mybir.ActivationFunctionType.Exp,
                   mybir.ActivationFunctionType.Ln)
        combined = [k for k, v in t.items() if exp in v and ln in v]
        if combined:
            for k, v in t.items():
                if k not in combined:
                    v.discard(exp)
                    v.discard(ln)
        return t

    bacc.get_activation_tables = patched
    return lambda: setattr(bacc, "get_activation_tables", orig)


def _build_program():
    import contextlib

    import concourse.bass as bass
    import concourse.mybir as mybir
    import concourse.tile as tile
    from concourse import bacc

    unpatch = _patch_act_tables()
    nc = bacc.Bacc("TRN2", target_bir_lowering=False, debug=False, num_devices=8)
    f32 = mybir.dt.float32
    bf16 = mybir.dt.bfloat16
    aps = {
        "x": nc.dram_tensor("x", [T, C], bf16, kind="ExternalInput").ap(),
        "wqkv": nc.dram_tensor("wqkv", [C, 3 * GQ], bf16, kind="ExternalInput").ap(),
        "bqk": nc.dram_tensor("bqk", [P, 8], f32, kind="ExternalInput").ap(),
        "bv": nc.dram_tensor("bv", [GQ], f32, kind="ExternalInput").ap(),
        "wp": nc.dram_tensor("wp", [GQ, C], bf16, kind="ExternalInput").ap(),
        "yT": nc.dram_tensor("yT", [C, T], bf16, kind="ExternalOutput").ap(),
    }
    with tile.TileContext(nc) as tc:
        with contextlib.ExitStack() as ctx:
            _emit(ctx, tc, aps, mybir, bass)
    nc.compile()
    unpatch()
    return nc


def get_program():
    global _PROGRAM
    if _PROGRAM is None:
        _PROGRAM = _build_program()
    return _PROGRAM


def make_in_maps(x, w_qkv, b_qkv, w_proj):
    import ml_dtypes

    bf16 = ml_dtypes.bfloat16
    x = np.asarray(x, np.float32)
    w_qkv = np.asarray(w_qkv, np.float32)
    b_qkv = np.asarray(b_qkv, np.float32)
    w_proj = np.asarray(w_proj, np.float32)
    in_maps = []
    for c in range(8):
        b = c // 2
        g = c % 2
        q0 = g * GQ
        wq = w_qkv[:, q0:q0 + GQ]
        wk = w_qkv[:, C + q0:C + q0 + GQ]
        wv = w_qkv[:, 2 * C + q0:2 * C + q0 + GQ]
        wqkv = np.ascontiguousarray(
            np.concatenate([wq, wk, wv], axis=1).astype(bf16))
        bq = b_qkv[q0:q0 + GQ]
        bk = b_qkv[C + q0:C + q0 + GQ]
        bqk = np.ascontiguousarray(np.concatenate([bq, bk]).reshape(8, P).T)
        bv = np.ascontiguousarray(b_qkv[2 * C + q0:2 * C + q0 + GQ])
        in_maps.append({
            "x": np.ascontiguousarray(x[b].astype(bf16)),
            "wqkv": wqkv,
            "bqk": bqk,
            "bv": bv,
            "wp": np.ascontiguousarray(
                w_proj[q0:q0 + GQ, :].astype(bf16)),
        })
    return in_maps


def combine_outputs(outs, b_proj):
    b_proj = np.asarray(b_proj, np.float32)
    y = np.empty((B, T, C), np.float32)
    for b in range(B):
        acc = (outs[2 * b].astype(np.float32)
               + outs[2 * b + 1].astype(np.float32))  # [C, T]
        y[b] = acc.T + b_proj
    return y


def kernel(x, w_qkv, b_qkv, w_proj, b_proj, _trace=False):
    from concourse import bass_utils

    nc = get_program()
    in_maps = make_in_maps(x, w_qkv, b_qkv, w_proj)
    res = bass_utils.run_bass_kernel_spmd(
        nc, in_maps, core_ids=list(range(8)), trace=_trace
    )
    outs = [r["yT"] for r in res.results]
    y = combine_outputs(outs, b_proj)
    if _trace:
        return y, res
    return y


# revision 45
# speedup vs baseline: 1.1247x; 1.1247x over previous
"""Causal self-attention on 8 TRN2 NeuronCores.

Sharding: core c handles batch b = c//2 and head-group g = c%2 (8 of 16 heads).
Each core computes its partial y^T = w_proj[slice].T @ o^T (contraction over its
512 o-channels); the host sums the two partials per batch and adds b_proj.

All matmuls run in bf16 (f32 psum accumulate). The attention pipeline is
software-pipelined (PV lags QK by one chunk) and qkv/proj matmuls are pumped
into the attention stream as fillers so the PE never idles (keeps HAM warm).

Shapes (hardcoded): B=4, T=2048, C=1024, H=16, HD=64.
"""

import numpy as np

B, T, C, H = 4, 2048, 1024, 16
HD = C // H          # 64
G = 2                # head groups
NHL = H // G         # 8 heads per core
GQ = NHL * HD        # 512 channel slice per core
P = 128
NT = T // P          # 16 token tiles / k-chunks
NCHUNK = C // P      # 8 contraction chunks for qkv
SCALE = 1.0 / float(np.sqrt(HD))

_PROGRAM = None


def _emit(ctx, tc, aps, mybir, bass):
    nc = tc.nc
    f32 = mybir.dt.float32
    bf16 = mybir.dt.bfloat16
    EXP = mybir.ActivationFunctionType.Exp
    LN = mybir.ActivationFunctionType.Ln

    x_d, wqkv_d, bqk_d, bv_d, wp_d, yT_d = (
        aps["x"], aps["wqkv"], aps["bqk"], aps["bv"], aps["wp"], aps["yT"],
    )

    # ---------------- pools ----------------
    const = ctx.enter_context(tc.tile_pool(name="const", bufs=1))
    # psum: sc 2x[128,1024] (4 banks) + pv 1x[128,1024] (2 banks)
    #       + fillers 2x[128,512] (2 banks) = 8 banks exactly
    scp = ctx.enter_context(tc.tile_pool(name="scp", bufs=2, space="PSUM"))
    pvp = ctx.enter_context(tc.tile_pool(name="pvp", bufs=1, space="PSUM"))
    filp = ctx.enter_context(tc.tile_pool(name="filp", bufs=2, space="PSUM"))

    xTp = ctx.enter_context(tc.tile_pool(name="xTp", bufs=8))
    qkp = ctx.enter_context(tc.tile_pool(name="qkp", bufs=8))
    vap = ctx.enter_context(tc.tile_pool(name="vap", bufs=16))
    oTp = ctx.enter_context(tc.tile_pool(name="oTp", bufs=4))
    ptp = ctx.enter_context(tc.tile_pool(name="ptp", bufs=4))
    evp = ctx.enter_context(tc.tile_pool(name="evp", bufs=2))
    rcp = ctx.enter_context(tc.tile_pool(name="rcp", bufs=2))
    wqkp = ctx.enter_context(tc.tile_pool(name="wqkp", bufs=4))
    wvp = ctx.enter_context(tc.tile_pool(name="wvp", bufs=1))
    wpp = ctx.enter_context(tc.tile_pool(name="wpp", bufs=1))
    xp = ctx.enter_context(tc.tile_pool(name="xp", bufs=6))
    ysp = ctx.enter_context(tc.tile_pool(name="ysp", bufs=3))

    # constants / weights
    identity = const.tile([P, P], bf16)
    from concourse.masks import make_identity
    make_identity(nc, identity)

    wqkv_r = wqkv_d.rearrange("(a p) n -> p a n", p=P)  # [128, 8, 1536]
    wqk_tiles = {}

    def load_wqk(ct):
        if ct in wqk_tiles:
            return
        w_t = wqkp.tile([P, NCHUNK, P], bf16, name=f"wqk{ct}", tag="wqk")
        nc.sync.dma_start(w_t[:], wqkv_r[:, :, ct * P:(ct + 1) * P])
        wqk_tiles[ct] = w_t

    # persistent SBUF tensors
    xT = []  # 8 x [128 c, 2048 t] bf16
    for r in range(NCHUNK):
        t_ = xTp.tile([P, T], bf16, name=f"xT{r}", tag="xT")
        xT.append(t_)
    qkT = []  # [128 c', 2048 t] bf16; 0..3 = qT, 4..7 = kT
    for ct in range(8):
        o_t = qkp.tile([P, T], bf16, name=f"qkT{ct}", tag="qkT")
        qkT.append(o_t)
    ones8 = const.tile([P, NHL, 1], f32)
    nc.vector.memset(ones8[:], 1.0)
    vaug = []  # [128 k, 8 heads, 65] bf16 per k-chunk (col 64 = ones)
    for t in range(NT):
        va = vap.tile([P, NHL, HD + 1], bf16, name=f"vaug{t}", tag="vaug")
        nc.vector.tensor_copy(va[:, :, HD:HD + 1], ones8[:])
        vaug.append(va)
    oT = []  # per pair: [128 d (2 heads), 2048 q] bf16
    for hp in range(4):
        o_t = oTp.tile([P, T], bf16, name=f"oT{hp}", tag="oT")
        oT.append(o_t)

    # ---------------- phase A: load x, build xT (bf16) ----------------
    # x DMAs are emitted before any weight DMA so the PE isn't starved at
    # kernel start; weights stream in behind them.
    xtiles = []
    for t in range(NT):
        x_t = xp.tile([P, C], bf16, name=f"x_{t}", tag="x")
        nc.sync.dma_start(x_t[:], x_d[t * P:(t + 1) * P, :])
        xtiles.append(x_t)

    bqk_sb = const.tile([P, 8], f32)
    nc.sync.dma_start(bqk_sb[:], bqk_d[:])
    bvb = const.tile([P, GQ], f32)
    nc.sync.dma_start(bvb[:], bv_d[None, :].to_broadcast((P, GQ)))
    load_wqk(0)
    load_wqk(4)
    wv_t = wvp.tile([P, NCHUNK, GQ], bf16)
    nc.sync.dma_start(wv_t[:], wqkv_r[:, :, 2 * GQ:3 * GQ])

# phase A emitted below, interleaved with the first qkv units

    # ---------------- filler units (qkv / proj matmuls) ----------------
    work = []  # FIFO of closures, each ~1 matmul of N=512
    vq = []    # gated FIFO of (pair, wmin, closure), reserved per pair

    def pump(n, hp=None):
        for _ in range(n):
            if vq and vq[0][0] == hp:
                vq.pop(0)[2]()
            elif work:
                work.pop(0)()
            elif vq:
                vq.pop(0)[2]()

    def vdrain(hp, upto_w):
        while vq and (vq[0][0] < hp
                      or (vq[0][0] == hp and vq[0][1] <= upto_w)):
            vq.pop(0)[2]()

    def drain():
        while vq:
            vq.pop(0)[2]()
        while work:
            work.pop(0)()

    # Filler closures are 1 matmul of N=512 (~213ns warm); each unit
    # accumulates 8 (or 4) of them into one filp psum tile.
    def qk_unit(ct, twp, sw):
        cell = {}
        c0 = twp * 1024 + sw * 512

        def mk(a):
            def f():
                if a == 0:
                    cell["ps"] = filp.tile(
                        [P, 512], f32, name=f"fq{ct}_{twp}_{sw}", tag="fil")
                nc.tensor.matmul(
                    cell["ps"][:], wqk_tiles[ct][:, a, :],
                    xT[a][:, c0:c0 + 512],
                    start=(a == 0), stop=(a == NCHUNK - 1),
                )
                if a == NCHUNK - 1:
                    nc.vector.tensor_scalar_add(
                        qkT[ct][:, c0:c0 + 512], cell["ps"][:],
                        bqk_sb[:, ct:ct + 1])
            return f
        return [mk(a) for a in range(NCHUNK)]

    def v_unit(t):
        cell = {}

        def mk(a):
            def f():
                if a == 0:
                    cell["ps"] = filp.tile(
                        [P, GQ], f32, name=f"fv{t}", tag="fil")
                nc.tensor.matmul(
                    cell["ps"][:], xT[a][:, t * P:(t + 1) * P], wv_t[:, a, :],
                    start=(a == 0), stop=(a == NCHUNK - 1),
                )
                if a == NCHUNK - 1:
                    nc.vector.tensor_add(
                        vaug[t][:, :, 0:HD],
                        cell["ps"][:].rearrange("p (h d) -> p h d", h=NHL),
                        bvb[:].rearrange("p (h d) -> p h d", h=NHL),
                    )
            return f
        return [mk(a) for a in range(NCHUNK)]

    def proj_unit(mt, w):
        cell = {}

        def mk(a):
            def f():
                if a == 0:
                    cell["ps"] = filp.tile(
                        [P, 512], f32, name=f"fp{mt}_{w}", tag="fil")
                nc.tensor.matmul(
                    cell["ps"][:], wp_t[:, a, mt * P:(mt + 1) * P],
                    oT[a][:, w * 512:(w + 1) * 512],
                    start=(a == 0), stop=(a == 3),
                )
                if a == 3:
                    ys = ysp.tile([P, 512], bf16, name=f"ys{mt}_{w}", tag="ys")
                    nc.vector.tensor_copy(ys[:], cell["ps"][:])
                    nc.sync.dma_start(
                        yT_d[mt * P:(mt + 1) * P, w * 512:(w + 1) * 512],
                        ys[:])
            return f
        return [mk(a) for a in range(4)]

    # pre-phase: transposes interleaved with pair-0 q/k and v units at
    # 512-token granularity, so the PE has work as soon as the first x
    # tiles land instead of idling through the DMA stream.
    def transpose_tg(tg):
        xts = [xtiles[2 * tg], xtiles[2 * tg + 1]]
        tp = scp.tile([P, 2048], bf16, name=f"tp_{tg}", tag="main")
        for r in range(NCHUNK):
            for tt in range(2):
                nc.tensor.transpose(
                    tp[:, r * 256 + tt * P: r * 256 + (tt + 1) * P],
                    xts[tt][:, r * P:(r + 1) * P],
                    identity,
                )
        for r in range(NCHUNK):
            nc.vector.tensor_copy(
                xT[r][:, tg * 256:(tg + 1) * 256],
                tp[:, r * 256:(r + 1) * 256],
            )

    for p in range(4):  # 512-token spans
        transpose_tg(2 * p)
        transpose_tg(2 * p + 1)
        for ct in (0, 4):
            for f in qk_unit(ct, p // 2, p % 2):
                f()
        for t in range(4 * p, 4 * p + 4):
            for f in v_unit(t):
                f()

    # ---------------- attention (software-pipelined) ----------------
    def attn_pair(hp, on_window=None, pre_window=None):
        qt = qkT[hp]
        kt = qkT[4 + hp]
        chunks = []
        for m in range(4):
            for i in range(4 * m + 4):
                chunks.append((m, i))
        pvt = {}   # window -> psum tile
        pts = {}   # idx -> pt tile

        def emit_qk(idx):
            m, i = chunks[idx]
            ws = m * 512
            s = max(i * P, ws)
            o = s - ws
            sc = scp.tile([P, 1024], f32, name=f"sc_{hp}_{m}_{i}", tag="main")
            for hh in range(2):
                r0 = hh * HD
                c0 = o if hh == 0 else 512
                nc.tensor.matmul(
                    sc[:, c0:c0 + 512 - o],
                    kt[r0:r0 + HD, i * P:(i + 1) * P],
                    qt[r0:r0 + HD, s:ws + 512],
                    start=True, stop=True,
                )
            pt = ptp.tile([P, 1024], bf16, name=f"pt_{hp}_{m}_{i}", tag="pt")
            nc.scalar.activation(pt[:, o:1024 - o], sc[:, o:1024 - o],
                                 EXP, scale=SCALE)
            if i * P >= ws:  # diagonal chunk: causal mask inside the block
                for hh in range(2):
                    c0 = o if hh == 0 else 512
                    nc.gpsimd.affine_select(
                        out=pt[:, c0:c0 + P],
                        in_=pt[:, c0:c0 + P],
                        compare_op=mybir.AluOpType.is_ge,
                        fill=0.0,
                        base=0,
                        pattern=[[1, P]],
                        channel_multiplier=-1,
                    )
            pts[idx] = pt

        def emit_pv(idx):
            m, i = chunks[idx]
            ws = m * 512
            o = max(i * P, ws) - ws
            if m not in pvt:
                pvt[m] = pvp.tile([P, 1024], f32, name=f"pv_{hp}_{m}",
                                  tag="pv")
            pt = pts.pop(idx)
            for hh in range(2):
                c0 = o if hh == 0 else 512
                nc.tensor.matmul(
                    pvt[m][0:HD + 1, hh * 512 + o:(hh + 1) * 512],
                    vaug[i][:, 2 * hp + hh, :],
                    pt[:, c0:c0 + 512 - o],
                    start=(i == 0), stop=(i == 4 * m + 3),
                )
            if i == 4 * m + 3:
                normalize(m)

        def normalize(m):
            ws = m * 512
            ev = evp.tile([P, 1024], f32, name=f"ev_{hp}_{m}", tag="ev")
            nc.vector.tensor_copy(ev[0:HD + 1, :], pvt[m][0:HD + 1, :])
            del pvt[m]
            # 1/d = exp(-ln d) on ACT (one pinned table): no DMA hops and no
            # DVE-FIFO blockage; broadcast+muls ride the gpsimd FIFO.
            lg = rcp.tile([1, 1024], f32, name=f"lg_{hp}_{m}", tag="lg")
            nc.scalar.activation(lg[:], ev[HD:HD + 1, :], LN)
            rc = rcp.tile([1, 1024], f32, name=f"rc_{hp}_{m}", tag="rc")
            nc.scalar.activation(rc[:], lg[:], EXP, scale=-1.0)
            rcb = rcp.tile([HD, 1024], f32, name=f"rcb_{hp}_{m}", tag="rcb")
            nc.gpsimd.partition_broadcast(rcb[:], rc[:])
            for hh in range(2):
                nc.gpsimd.tensor_mul(
                    oT[hp][hh * HD:(hh + 1) * HD, ws:ws + 512],
                    ev[0:HD, hh * 512:(hh + 1) * 512],
                    rcb[:, hh * 512:(hh + 1) * 512],
                )
            if on_window is not None:
                on_window(m)

        for idx in range(len(chunks)):
            if chunks[idx][1] == 0 and pre_window is not None:
                pre_window(chunks[idx][0])
            emit_qk(idx)
            if idx >= 1:
                # overstuff the PE so it lags ACT: deps at the FIFO head are
                # then long-resolved -> no micro-stalls -> HAM stays warm
                pump(4 if chunks[idx][1] == 0 else (2 if idx % 3 == 0 else 1), hp)
                emit_pv(idx - 1)
        pump(2, hp)
        emit_pv(len(chunks) - 1)

    def chase(m):
        # pair 3 finished window m -> its proj columns are now computable;
        # queue them as fillers for pair 3's remaining windows
        for mt in range(NCHUNK):
            work.extend(proj_unit(mt, m))

    for hp in range(4):
        if hp < 3:
            load_wqk(hp + 1)
            load_wqk(4 + hp + 1)
            if hp == 2:
                wp_t = wpp.tile([P, 4, C], bf16)
                nc.sync.dma_start(wp_t[:], wp_d.rearrange("(a p) n -> p a n", p=P))
            for ct in (hp + 1, 4 + hp + 1):
                for twp in range(2):
                    for sw in range(2):
                        span = 2 * twp + sw
                        if hp == 2 and span >= 1:
                            # defer pair-3 qk for later windows into pair 3
                            # itself (window m reads spans <= m only)
                            for f in qk_unit(ct, twp, sw):
                                vq.append((3, span, f))
                        else:
                            work.extend(qk_unit(ct, twp, sw))
            attn_pair(hp, pre_window=(lambda m, hp=hp: vdrain(hp, m)))
            # qk leftovers for pair hp+1's early windows must complete now
            while work:
                work.pop(0)()
        else:
            attn_pair(hp, on_window=chase,
                      pre_window=(lambda m: vdrain(3, m)))

    # pair 3 done; remaining proj (window 3 queued by chase(3) + leftovers)
    drain()


def _patch_act_tables():
    """Narrow the activation-table chooser so Exp and Ln resolve to the one
    table containing both (act indices/contents on HW are unchanged — this
    only removes lesser alternatives from the insert_act_table_loads pass,
    eliminating per-window table-reload thrash)."""
    import functools

    import concourse.bacc as bacc
    import concourse.hw_specs as hw_specs
    import concourse.mybir as mybir

    orig = hw_specs.get_activation_tables

    @functools.cache
    def patched(module_arch):
        t = {k: set(v) for k, v in orig(module_arch).items()}
        exp, ln = (mybir.ActivationFunctionType.Exp,
                   mybir.ActivationFunctionType.Ln)
        combined = [k for k, v in t.items() if exp in v and ln in v]
        if combined:
            for k, v in t.items():
                if k not in combined:
                    v.discard(exp)
                    v.discard(ln)
        return t

    bacc.get_activation_tables = patched
    return lambda: setattr(bacc, "get_activation_tables", orig)


def _build_program():
    import contextlib

    import concourse.bass as bass
    import concourse.mybir as mybir
    import concourse.tile as tile
    from concourse import bacc

    unpatch = _patch_act_tables()
    nc = bacc.Bacc("TRN2", target_bir_lowering=False, debug=False, num_devices=8)
    f32 = mybir.dt.float32
    bf16 = mybir.dt.bfloat16
    aps = {
        "x": nc.dram_tensor("x", [T, C], bf16, kind="ExternalInput").ap(),
        "wqkv": nc.dram_tensor("wqkv", [C, 3 * GQ], bf16, kind="ExternalInput").ap(),
        "bqk": nc.dram_tensor("bqk", [P, 8], f32, kind="ExternalInput").ap(),
        "bv": nc.dram_tensor("bv", [GQ], f32, kind="ExternalInput").ap(),
        "wp": nc.dram_tensor("wp", [GQ, C], bf16, kind="ExternalInput").ap(),
        "yT": nc.dram_tensor("yT", [C, T], bf16, kind="ExternalOutput").ap(),
    }
    with tile.TileContext(nc) as tc:
        with contextlib.ExitStack() as ctx:
            _emit(ctx, tc, aps, mybir, bass)
    nc.compile()
    unpatch()
    return nc


def get_program():
    global _PROGRAM
    if _PROGRAM is None:
        _PROGRAM = _build_program()
    return _PROGRAM


def make_in_maps(x, w_qkv, b_qkv, w_proj):
    import ml_dtypes

    bf16 = ml_dtypes.bfloat16
    x = np.asarray(x, np.float32)
    w_qkv = np.asarray(w_qkv, np.float32)
    b_qkv = np.asarray(b_qkv, np.float32)
    w_proj = np.asarray(w_proj, np.float32)
    in_maps = []
    for c in range(8):
        b = c // 2
        g = c % 2
        q0 = g * GQ
        wq = w_qkv[:, q0:q0 + GQ]
        wk = w_qkv[:, C + q0:C + q0 + GQ]
        wv = w_qkv[:, 2 * C + q0:2 * C + q0 + GQ]
        wqkv = np.ascontiguousarray(
            np.concatenate([wq, wk, wv], axis=1).astype(bf16))
        bq = b_qkv[q0:q0 + GQ]
        bk = b_qkv[C + q0:C + q0 + GQ]
        bqk = np.ascontiguousarray(np.concatenate([bq, bk]).reshape(8, P).T)
        bv = np.ascontiguousarray(b_qkv[2 * C + q0:2 * C + q0 + GQ])
        in_maps.append({
            "x": np.ascontiguousarray(x[b].astype(bf16)),
            "wqkv": wqkv,
            "bqk": bqk,
            "bv": bv,
            "wp": np.ascontiguousarray(
                w_proj[q0:q0 + GQ, :].astype(bf16)),
        })
    return in_maps


def combine_outputs(outs, b_proj):
    b_proj = np.asarray(b_proj, np.float32)
    y = np.empty((B, T, C), np.float32)
    for b in range(B):
        acc = (outs[2 * b].astype(np.float32)
               + outs[2 * b + 1].astype(np.float32))  # [C, T]
        y[b] = acc.T + b_proj
    return y


def kernel(x, w_qkv, b_qkv, w_proj, b_proj, _trace=False):
    from concourse import bass_utils

    nc = get_program()
    in_maps = make_in_maps(x, w_qkv, b_qkv, w_proj)
    res = bass_utils.run_bass_kernel_spmd(
        nc, in_maps, core_ids=list(range(8)), trace=_trace
    )
    outs = [r["yT"] for r in res.results]
    y = combine_outputs(outs, b_proj)
    if _trace:
        return y, res
    return y


# revision 46
# speedup vs baseline: 1.5891x; 1.4129x over previous
"""Causal self-attention on 8 TRN2 NeuronCores.

Sharding: core c handles batch b = c//2 and head-group g = c%2 (8 of 16 heads).
Each core computes its partial y^T = w_proj[slice].T @ o^T (contraction over its
512 o-channels); the host sums the two partials per batch and adds b_proj.

All matmuls run in bf16 (f32 psum accumulate). The attention pipeline is
software-pipelined (PV lags QK by one chunk) and qkv/proj matmuls are pumped
into the attention stream as fillers so the PE never idles (keeps HAM warm).

Shapes (hardcoded): B=4, T=2048, C=1024, H=16, HD=64.
"""

import numpy as np

B, T, C, H = 4, 2048, 1024, 16
HD = C // H          # 64
G = 2                # head groups
NHL = H // G         # 8 heads per core
GQ = NHL * HD        # 512 channel slice per core
P = 128
NT = T // P          # 16 token tiles / k-chunks
NCHUNK = C // P      # 8 contraction chunks for qkv
SCALE = 1.0 / float(np.sqrt(HD))

_PROGRAM = None


def _emit(ctx, tc, aps, mybir, bass):
    nc = tc.nc
    f32 = mybir.dt.float32
    bf16 = mybir.dt.bfloat16
    EXP = mybir.ActivationFunctionType.Exp
    LN = mybir.ActivationFunctionType.Ln

    x_d, wqkv_d, bqk_d, bv_d, wp_d, yT_d = (
        aps["x"], aps["wqkv"], aps["bqk"], aps["bv"], aps["wp"], aps["yT"],
    )

    # ---------------- pools ----------------
    const = ctx.enter_context(tc.tile_pool(name="const", bufs=1))
    # psum: sc 2x[128,1024] (4 banks) + pv 1x[128,1024] (2 banks)
    #       + fillers 2x[128,512] (2 banks) = 8 banks exactly
    scp = ctx.enter_context(tc.tile_pool(name="scp", bufs=2, space="PSUM"))
    pvp = ctx.enter_context(tc.tile_pool(name="pvp", bufs=1, space="PSUM"))
    filp = ctx.enter_context(tc.tile_pool(name="filp", bufs=2, space="PSUM"))

    xTp = ctx.enter_context(tc.tile_pool(name="xTp", bufs=8))
    qkp = ctx.enter_context(tc.tile_pool(name="qkp", bufs=8))
    vap = ctx.enter_context(tc.tile_pool(name="vap", bufs=16))
    oTp = ctx.enter_context(tc.tile_pool(name="oTp", bufs=4))
    ptp = ctx.enter_context(tc.tile_pool(name="ptp", bufs=4))
    evp = ctx.enter_context(tc.tile_pool(name="evp", bufs=2))
    rcp = ctx.enter_context(tc.tile_pool(name="rcp", bufs=2))
    wqkp = ctx.enter_context(tc.tile_pool(name="wqkp", bufs=4))
    wvp = ctx.enter_context(tc.tile_pool(name="wvp", bufs=1))
    wpp = ctx.enter_context(tc.tile_pool(name="wpp", bufs=1))
    xp = ctx.enter_context(tc.tile_pool(name="xp", bufs=6))
    ysp = ctx.enter_context(tc.tile_pool(name="ysp", bufs=3))

    # constants / weights
    identity = const.tile([P, P], bf16)
    from concourse.masks import make_identity
    make_identity(nc, identity)

    wqkv_r = wqkv_d.rearrange("(a p) n -> p a n", p=P)  # [128, 8, 1536]
    wqk_tiles = {}

    def load_wqk(ct):
        if ct in wqk_tiles:
            return
        w_t = wqkp.tile([P, NCHUNK, P], bf16, name=f"wqk{ct}", tag="wqk")
        nc.sync.dma_start(w_t[:], wqkv_r[:, :, ct * P:(ct + 1) * P])
        wqk_tiles[ct] = w_t

    # persistent SBUF tensors
    xT = []  # 8 x [128 c, 2048 t] bf16
    for r in range(NCHUNK):
        t_ = xTp.tile([P, T], bf16, name=f"xT{r}", tag="xT")
        xT.append(t_)
    qkT = []  # [128 c', 2048 t] bf16; 0..3 = qT, 4..7 = kT
    for ct in range(8):
        o_t = qkp.tile([P, T], bf16, name=f"qkT{ct}", tag="qkT")
        qkT.append(o_t)
    ones8 = const.tile([P, NHL, 1], f32)
    nc.vector.memset(ones8[:], 1.0)
    vaug = []  # [128 k, 8 heads, 65] bf16 per k-chunk (col 64 = ones)
    for t in range(NT):
        va = vap.tile([P, NHL, HD + 1], bf16, name=f"vaug{t}", tag="vaug")
        nc.vector.tensor_copy(va[:, :, HD:HD + 1], ones8[:])
        vaug.append(va)
    oT = []  # per pair: [128 d (2 heads), 2048 q] bf16
    for hp in range(4):
        o_t = oTp.tile([P, T], bf16, name=f"oT{hp}", tag="oT")
        oT.append(o_t)

    # ---------------- phase A: load x, build xT (bf16) ----------------
    # x DMAs are emitted before any weight DMA so the PE isn't starved at
    # kernel start; weights stream in behind them.
    xtiles = []
    for t in range(NT):
        x_t = xp.tile([P, C], bf16, name=f"x_{t}", tag="x")
        nc.sync.dma_start(x_t[:], x_d[t * P:(t + 1) * P, :])
        xtiles.append(x_t)

    bqk_sb = const.tile([P, 8], f32)
    nc.sync.dma_start(bqk_sb[:], bqk_d[:])
    bvb = const.tile([P, GQ], f32)
    nc.sync.dma_start(bvb[:], bv_d[None, :].to_broadcast((P, GQ)))
    load_wqk(0)
    load_wqk(4)
    wv_t = wvp.tile([P, NCHUNK, GQ], bf16)
    nc.sync.dma_start(wv_t[:], wqkv_r[:, :, 2 * GQ:3 * GQ])

# phase A emitted below, interleaved with the first qkv units

    # ---------------- filler units (qkv / proj matmuls) ----------------
    work = []  # FIFO of closures, each ~1 matmul of N=512
    vq = []    # gated FIFO of (pair, wmin, closure), reserved per pair

    def pump(n, hp=None):
        for _ in range(n):
            if vq and vq[0][0] == hp:
                vq.pop(0)[2]()
            elif work:
                work.pop(0)()
            elif vq:
                vq.pop(0)[2]()

    def vdrain(hp, upto_w):
        while vq and (vq[0][0] < hp
                      or (vq[0][0] == hp and vq[0][1] <= upto_w)):
            vq.pop(0)[2]()

    def drain():
        while vq:
            vq.pop(0)[2]()
        while work:
            work.pop(0)()

    # Filler closures are 1 matmul of N=512 (~213ns warm); each unit
    # accumulates 8 (or 4) of them into one filp psum tile.
    def qk_unit(ct, twp, sw):
        cell = {}
        c0 = twp * 1024 + sw * 512

        def mk(a):
            def f():
                if a == 0:
                    cell["ps"] = filp.tile(
                        [P, 512], f32, name=f"fq{ct}_{twp}_{sw}", tag="fil")
                nc.tensor.matmul(
                    cell["ps"][:], wqk_tiles[ct][:, a, :],
                    xT[a][:, c0:c0 + 512],
                    start=(a == 0), stop=(a == NCHUNK - 1),
                )
                if a == NCHUNK - 1:
                    nc.vector.tensor_scalar_add(
                        qkT[ct][:, c0:c0 + 512], cell["ps"][:],
                        bqk_sb[:, ct:ct + 1])
            return f
        return [mk(a) for a in range(NCHUNK)]

    def v_unit(t):
        cell = {}

        def mk(a):
            def f():
                if a == 0:
                    cell["ps"] = filp.tile(
                        [P, GQ], f32, name=f"fv{t}", tag="fil")
                nc.tensor.matmul(
                    cell["ps"][:], xT[a][:, t * P:(t + 1) * P], wv_t[:, a, :],
                    start=(a == 0), stop=(a == NCHUNK - 1),
                )
                if a == NCHUNK - 1:
                    nc.vector.tensor_add(
                        vaug[t][:, :, 0:HD],
                        cell["ps"][:].rearrange("p (h d) -> p h d", h=NHL),
                        bvb[:].rearrange("p (h d) -> p h d", h=NHL),
                    )
            return f
        return [mk(a) for a in range(NCHUNK)]

    def proj_unit(mt, w):
        cell = {}

        def mk(a):
            def f():
                if a == 0:
                    cell["ps"] = filp.tile(
                        [P, 512], f32, name=f"fp{mt}_{w}", tag="fil")
                nc.tensor.matmul(
                    cell["ps"][:], wp_t[:, a, mt * P:(mt + 1) * P],
                    oT[a][:, w * 512:(w + 1) * 512],
                    start=(a == 0), stop=(a == 3),
                )
                if a == 3:
                    ys = ysp.tile([P, 512], bf16, name=f"ys{mt}_{w}", tag="ys")
                    nc.vector.tensor_copy(ys[:], cell["ps"][:])
                    nc.sync.dma_start(
                        yT_d[mt * P:(mt + 1) * P, w * 512:(w + 1) * 512],
                        ys[:])
            return f
        return [mk(a) for a in range(4)]

    # pre-phase: transposes interleaved with pair-0 q/k and v units at
    # 512-token granularity, so the PE has work as soon as the first x
    # tiles land instead of idling through the DMA stream.
    def transpose_tg(tg):
        xts = [xtiles[2 * tg], xtiles[2 * tg + 1]]
        tp = scp.tile([P, 2048], bf16, name=f"tp_{tg}", tag="main")
        for r in range(NCHUNK):
            for tt in range(2):
                nc.tensor.transpose(
                    tp[:, r * 256 + tt * P: r * 256 + (tt + 1) * P],
                    xts[tt][:, r * P:(r + 1) * P],
                    identity,
                )
        for r in range(NCHUNK):
            nc.vector.tensor_copy(
                xT[r][:, tg * 256:(tg + 1) * 256],
                tp[:, r * 256:(r + 1) * 256],
            )

    for p in range(4):  # 512-token spans
        transpose_tg(2 * p)
        transpose_tg(2 * p + 1)
        for ct in (0, 4):
            for f in qk_unit(ct, p // 2, p % 2):
                f()
        for t in range(4 * p, 4 * p + 4):
            for f in v_unit(t):
                f()

    # ---------------- attention (software-pipelined) ----------------
    def attn_pair(hp, on_window=None, pre_window=None):
        qt = qkT[hp]
        kt = qkT[4 + hp]
        chunks = []
        for m in range(4):
            for i in range(4 * m + 4):
                chunks.append((m, i))
        pvt = {}   # window -> psum tile
        pts = {}   # idx -> pt tile
        state = {}  # window -> (ev, rcb) awaiting deferred muls
        deferred = []

        def emit_qk(idx):
            m, i = chunks[idx]
            ws = m * 512
            s = max(i * P, ws)
            o = s - ws
            sc = scp.tile([P, 1024], f32, name=f"sc_{hp}_{m}_{i}", tag="main")
            for hh in range(2):
                r0 = hh * HD
                c0 = o if hh == 0 else 512
                nc.tensor.matmul(
                    sc[:, c0:c0 + 512 - o],
                    kt[r0:r0 + HD, i * P:(i + 1) * P],
                    qt[r0:r0 + HD, s:ws + 512],
                    start=True, stop=True,
                )
            pt = ptp.tile([P, 1024], bf16, name=f"pt_{hp}_{m}_{i}", tag="pt")
            nc.scalar.activation(pt[:, o:1024 - o], sc[:, o:1024 - o],
                                 EXP, scale=SCALE)
            if i * P >= ws:  # diagonal chunk: causal mask inside the block
                for hh in range(2):
                    c0 = o if hh == 0 else 512
                    nc.gpsimd.affine_select(
                        out=pt[:, c0:c0 + P],
                        in_=pt[:, c0:c0 + P],
                        compare_op=mybir.AluOpType.is_ge,
                        fill=0.0,
                        base=0,
                        pattern=[[1, P]],
                        channel_multiplier=-1,
                    )
            pts[idx] = pt

        def emit_pv(idx):
            m, i = chunks[idx]
            ws = m * 512
            o = max(i * P, ws) - ws
            if m not in pvt:
                pvt[m] = pvp.tile([P, 1024], f32, name=f"pv_{hp}_{m}",
                                  tag="pv")
            pt = pts.pop(idx)
            for hh in range(2):
                c0 = o if hh == 0 else 512
                nc.tensor.matmul(
                    pvt[m][0:HD + 1, hh * 512 + o:(hh + 1) * 512],
                    vaug[i][:, 2 * hp + hh, :],
                    pt[:, c0:c0 + 512 - o],
                    start=(i == 0), stop=(i == 4 * m + 3),
                )
            if i == 4 * m + 3:
                normalize(m)

        def normalize(m):
            ws = m * 512
            ev = evp.tile([P, 1024], f32, name=f"ev_{hp}_{m}", tag="ev")
            nc.vector.tensor_copy(ev[0:HD + 1, :], pvt[m][0:HD + 1, :])
            del pvt[m]
            # 1/d = exp(-ln d) on ACT (one pinned table): no DMA hops and no
            # DVE-FIFO blockage; broadcast+muls ride the gpsimd FIFO.
            lg = rcp.tile([1, 1024], f32, name=f"lg_{hp}_{m}", tag="lg")
            nc.scalar.activation(lg[:], ev[HD:HD + 1, :], LN)
            rc = rcp.tile([1, 1024], f32, name=f"rc_{hp}_{m}", tag="rc")
            nc.scalar.activation(rc[:], lg[:], EXP, scale=-1.0)
            rcb = rcp.tile([HD, 1024], f32, name=f"rcb_{hp}_{m}", tag="rcb")
            nc.gpsimd.partition_broadcast(rcb[:], rc[:])
            state[m] = (ev, rcb)

        def muls(m):
            # deferred ~3 chunks after normalize(m): rcb is ready by then, so
            # these never block the DVE FIFO (which recycles filler psum)
            ws = m * 512
            ev, rcb = state.pop(m)
            for hh in range(2):
                nc.vector.tensor_mul(
                    oT[hp][hh * HD:(hh + 1) * HD, ws:ws + 512],
                    ev[0:HD, hh * 512:(hh + 1) * 512],
                    rcb[:, hh * 512:(hh + 1) * 512],
                )
            if on_window is not None:
                on_window(m)

        def emit_pv_s(idx):
            emit_pv(idx)
            m, i = chunks[idx]
            if i == 4 * m + 3:
                deferred.append((idx + 3, lambda m=m: muls(m)))

        for idx in range(len(chunks)):
            while deferred and deferred[0][0] <= idx:
                deferred.pop(0)[1]()
            if chunks[idx][1] == 0 and pre_window is not None:
                pre_window(chunks[idx][0])
            emit_qk(idx)
            if idx >= 1:
                # overstuff the PE so it lags ACT: deps at the FIFO head are
                # then long-resolved -> no micro-stalls -> HAM stays warm
                pump(4 if chunks[idx][1] == 0 else (2 if idx % 3 == 0 else 1), hp)
                emit_pv_s(idx - 1)
        pump(2, hp)
        emit_pv_s(len(chunks) - 1)
        while deferred:
            deferred.pop(0)[1]()

    def chase(m):
        # pair 3 finished window m -> its proj columns are now computable;
        # queue them as fillers for pair 3's remaining windows
        for mt in range(NCHUNK):
            work.extend(proj_unit(mt, m))

    for hp in range(4):
        if hp < 3:
            load_wqk(hp + 1)
            load_wqk(4 + hp + 1)
            if hp == 2:
                wp_t = wpp.tile([P, 4, C], bf16)
                nc.sync.dma_start(wp_t[:], wp_d.rearrange("(a p) n -> p a n", p=P))
            for ct in (hp + 1, 4 + hp + 1):
                for twp in range(2):
                    for sw in range(2):
                        span = 2 * twp + sw
                        if hp == 2 and span >= 1:
                            # defer pair-3 qk for later windows into pair 3
                            # itself (window m reads spans <= m only)
                            for f in qk_unit(ct, twp, sw):
                                vq.append((3, span, f))
                        else:
                            work.extend(qk_unit(ct, twp, sw))
            attn_pair(hp, pre_window=(lambda m, hp=hp: vdrain(hp, m)))
            # qk leftovers for pair hp+1's early windows must complete now
            while work:
                work.pop(0)()
        else:
            attn_pair(hp, on_window=chase,
                      pre_window=(lambda m: vdrain(3, m)))

    # pair 3 done; remaining proj (window 3 queued by chase(3) + leftovers)
    drain()


def _patch_act_tables():
    """Narrow the activation-table chooser so Exp and Ln resolve to the one
    table containing both (act indices/contents on HW are unchanged — this
    only removes lesser alternatives from the insert_act_table_loads pass,
    eliminating per-window table-reload thrash)."""
    import functools

    import concourse.bacc as bacc
    import concourse.hw_specs as hw_specs
    import concourse.mybir as mybir

    orig = hw_specs.get_activation_tables

    @functools.cache
    def patched(module_arch):
        t = {k: set(v) for k, v in orig(module_arch).items()}
        exp, ln = (mybir.ActivationFunctionType.Exp,
                   mybir.ActivationFunctionType.Ln)
        combined = [k for k, v in t.items() if exp in v and ln in v]
        if combined:
            for k, v in t.items():
                if k not in combined:
                    v.discard(exp)
                    v.discard(ln)
        return t

    bacc.get_activation_tables = patched
    return lambda: setattr(bacc, "get_activation_tables", orig)


def _build_program():
    import contextlib

    import concourse.bass as bass
    import concourse.mybir as mybir
    import concourse.tile as tile
    from concourse import bacc

    unpatch = _patch_act_tables()
    nc = bacc.Bacc("TRN2", target_bir_lowering=False, debug=False, num_devices=8)
    f32 = mybir.dt.float32
    bf16 = mybir.dt.bfloat16
    aps = {
        "x": nc.dram_tensor("x", [T, C], bf16, kind="ExternalInput").ap(),
        "wqkv": nc.dram_tensor("wqkv", [C, 3 * GQ], bf16, kind="ExternalInput").ap(),
        "bqk": nc.dram_tensor("bqk", [P, 8], f32, kind="ExternalInput").ap(),
        "bv": nc.dram_tensor("bv", [GQ], f32, kind="ExternalInput").ap(),
        "wp": nc.dram_tensor("wp", [GQ, C], bf16, kind="ExternalInput").ap(),
        "yT": nc.dram_tensor("yT", [C, T], bf16, kind="ExternalOutput").ap(),
    }
    with tile.TileContext(nc) as tc:
        with contextlib.ExitStack() as ctx:
            _emit(ctx, tc, aps, mybir, bass)
    nc.compile()
    unpatch()
    return nc


def get_program():
    global _PROGRAM
    if _PROGRAM is None:
        _PROGRAM = _build_program()
    return _PROGRAM


def make_in_maps(x, w_qkv, b_qkv, w_proj):
    import ml_dtypes

    bf16 = ml_dtypes.bfloat16
    x = np.asarray(x, np.float32)
    w_qkv = np.asarray(w_qkv, np.float32)
    b_qkv = np.asarray(b_qkv, np.float32)
    w_proj = np.asarray(w_proj, np.float32)
    in_maps = []
    for c in range(8):
        b = c // 2
        g = c % 2
        q0 = g * GQ
        wq = w_qkv[:, q0:q0 + GQ]
        wk = w_qkv[:, C + q0:C + q0 + GQ]
        wv = w_qkv[:, 2 * C + q0:2 * C + q0 + GQ]
        wqkv = np.ascontiguousarray(
            np.concatenate([wq, wk, wv], axis=1).astype(bf16))
        bq = b_qkv[q0:q0 + GQ]
        bk = b_qkv[C + q0:C + q0 + GQ]
        bqk = np.ascontiguousarray(np.concatenate([bq, bk]).reshape(8, P).T)
        bv = np.ascontiguousarray(b_qkv[2 * C + q0:2 * C + q0 + GQ])
        in_maps.append({
            "x": np.ascontiguousarray(x[b].astype(bf16)),
            "wqkv": wqkv,
            "bqk": bqk,
            "bv": bv,
            "wp": np.ascontiguousarray(
                w_proj[q0:q0 + GQ, :].astype(bf16)),
        })
    return in_maps


def combine_outputs(outs, b_proj):
    b_proj = np.asarray(b_proj, np.float32)
    y = np.empty((B, T, C), np.float32)
    for b in range(B):
        acc = (outs[2 * b].astype(np.float32)
               + outs[2 * b + 1].astype(np.float32))  # [C, T]
        y[b] = acc.T + b_proj
    return y


def kernel(x, w_qkv, b_qkv, w_proj, b_proj, _trace=False):
    from concourse import bass_utils

    nc = get_program()
    in_maps = make_in_maps(x, w_qkv, b_qkv, w_proj)
    res = bass_utils.run_bass_kernel_spmd(
        nc, in_maps, core_ids=list(range(8)), trace=_trace
    )
    outs = [r["yT"] for r in res.results]
    y = combine_outputs(outs, b_proj)
    if _trace:
        return y, res
    return y


# revision 47
# speedup vs baseline: 1.6639x; 1.0471x over previous
"""Causal self-attention on 8 TRN2 NeuronCores.

Sharding: core c handles batch b = c//2 and head-group g = c%2 (8 of 16 heads).
Each core computes its partial y^T = w_proj[slice].T @ o^T (contraction over its
512 o-channels); the host sums the two partials per batch and adds b_proj.

All matmuls run in bf16 (f32 psum accumulate). The attention pipeline is
software-pipelined (PV lags QK by one chunk) and qkv/proj matmuls are pumped
into the attention stream as fillers so the PE never idles (keeps HAM warm).

Shapes (hardcoded): B=4, T=2048, C=1024, H=16, HD=64.
"""

import numpy as np

B, T, C, H = 4, 2048, 1024, 16
HD = C // H          # 64
G = 2                # head groups
NHL = H // G         # 8 heads per core
GQ = NHL * HD        # 512 channel slice per core
P = 128
NT = T // P          # 16 token tiles / k-chunks
NCHUNK = C // P      # 8 contraction chunks for qkv
SCALE = 1.0 / float(np.sqrt(HD))

_PROGRAM = None


def _emit(ctx, tc, aps, mybir, bass):
    nc = tc.nc
    f32 = mybir.dt.float32
    bf16 = mybir.dt.bfloat16
    EXP = mybir.ActivationFunctionType.Exp
    LN = mybir.ActivationFunctionType.Ln

    x_d, wqkv_d, bqk_d, bv_d, wp_d, yT_d = (
        aps["x"], aps["wqkv"], aps["bqk"], aps["bv"], aps["wp"], aps["yT"],
    )

    # ---------------- pools ----------------
    const = ctx.enter_context(tc.tile_pool(name="const", bufs=1))
    # psum: sc 2x[128,1024] (4 banks) + pv 1x[128,1024] (2 banks)
    #       + fillers 2x[128,512] (2 banks) = 8 banks exactly
    scp = ctx.enter_context(tc.tile_pool(name="scp", bufs=2, space="PSUM"))
    pvp = ctx.enter_context(tc.tile_pool(name="pvp", bufs=1, space="PSUM"))
    filp = ctx.enter_context(tc.tile_pool(name="filp", bufs=2, space="PSUM"))

    xTp = ctx.enter_context(tc.tile_pool(name="xTp", bufs=8))
    qkp = ctx.enter_context(tc.tile_pool(name="qkp", bufs=8))
    vap = ctx.enter_context(tc.tile_pool(name="vap", bufs=16))
    oTp = ctx.enter_context(tc.tile_pool(name="oTp", bufs=4))
    ptp = ctx.enter_context(tc.tile_pool(name="ptp", bufs=4))
    evp = ctx.enter_context(tc.tile_pool(name="evp", bufs=2))
    rcp = ctx.enter_context(tc.tile_pool(name="rcp", bufs=2))
    wqkp = ctx.enter_context(tc.tile_pool(name="wqkp", bufs=4))
    wvp = ctx.enter_context(tc.tile_pool(name="wvp", bufs=1))
    wpp = ctx.enter_context(tc.tile_pool(name="wpp", bufs=1))
    xp = ctx.enter_context(tc.tile_pool(name="xp", bufs=6))
    ysp = ctx.enter_context(tc.tile_pool(name="ysp", bufs=3))

    # constants / weights
    identity = const.tile([P, P], bf16)
    from concourse.masks import make_identity
    make_identity(nc, identity)

    wqkv_r = wqkv_d.rearrange("(a p) n -> p a n", p=P)  # [128, 8, 1536]
    wqk_tiles = {}

    def load_wqk(ct):
        if ct in wqk_tiles:
            return
        w_t = wqkp.tile([P, NCHUNK, P], bf16, name=f"wqk{ct}", tag="wqk")
        nc.sync.dma_start(w_t[:], wqkv_r[:, :, ct * P:(ct + 1) * P])
        wqk_tiles[ct] = w_t

    # persistent SBUF tensors
    xT = []  # 8 x [128 c, 2048 t] bf16
    for r in range(NCHUNK):
        t_ = xTp.tile([P, T], bf16, name=f"xT{r}", tag="xT")
        xT.append(t_)
    qkT = []  # [128 c', 2048 t] bf16; 0..3 = qT, 4..7 = kT
    for ct in range(8):
        o_t = qkp.tile([P, T], bf16, name=f"qkT{ct}", tag="qkT")
        qkT.append(o_t)
    ones8 = const.tile([P, NHL, 1], f32)
    nc.vector.memset(ones8[:], 1.0)
    vaug = []  # [128 k, 8 heads, 65] bf16 per k-chunk (col 64 = ones)
    for t in range(NT):
        va = vap.tile([P, NHL, HD + 1], bf16, name=f"vaug{t}", tag="vaug")
        nc.vector.tensor_copy(va[:, :, HD:HD + 1], ones8[:])
        vaug.append(va)
    oT = []  # per pair: [128 d (2 heads), 2048 q] bf16
    for hp in range(4):
        o_t = oTp.tile([P, T], bf16, name=f"oT{hp}", tag="oT")
        oT.append(o_t)

    # ---------------- phase A: load x, build xT (bf16) ----------------
    # x DMAs are emitted before any weight DMA so the PE isn't starved at
    # kernel start; weights stream in behind them.
    xtiles = []
    for t in range(NT):
        x_t = xp.tile([P, C], bf16, name=f"x_{t}", tag="x")
        nc.sync.dma_start(x_t[:], x_d[t * P:(t + 1) * P, :])
        xtiles.append(x_t)

    bqk_sb = const.tile([P, 8], f32)
    nc.sync.dma_start(bqk_sb[:], bqk_d[:])
    bvb = const.tile([P, GQ], f32)
    nc.sync.dma_start(bvb[:], bv_d[None, :].to_broadcast((P, GQ)))
    load_wqk(0)
    load_wqk(4)
    wv_t = wvp.tile([P, NCHUNK, GQ], bf16)
    nc.sync.dma_start(wv_t[:], wqkv_r[:, :, 2 * GQ:3 * GQ])

# phase A emitted below, interleaved with the first qkv units

    # ---------------- filler units (qkv / proj matmuls) ----------------
    work = []  # FIFO of closures, each ~1 matmul of N=512
    vq = []    # gated FIFO of (pair, wmin, closure), reserved per pair

    def pump(n, hp=None):
        for _ in range(n):
            if vq and vq[0][0] == hp:
                vq.pop(0)[2]()
            elif work:
                work.pop(0)()
            elif vq:
                vq.pop(0)[2]()

    def vdrain(hp, upto_w):
        while vq and (vq[0][0] < hp
                      or (vq[0][0] == hp and vq[0][1] <= upto_w)):
            vq.pop(0)[2]()

    def drain():
        while vq:
            vq.pop(0)[2]()
        while work:
            work.pop(0)()

    # Filler closures are 1 matmul of N=512 (~213ns warm); each unit
    # accumulates 8 (or 4) of them into one filp psum tile.
    def qk_unit(ct, twp, sw):
        cell = {}
        c0 = twp * 1024 + sw * 512

        def mk(a):
            def f():
                if a == 0:
                    cell["ps"] = filp.tile(
                        [P, 512], f32, name=f"fq{ct}_{twp}_{sw}", tag="fil")
                nc.tensor.matmul(
                    cell["ps"][:], wqk_tiles[ct][:, a, :],
                    xT[a][:, c0:c0 + 512],
                    start=(a == 0), stop=(a == NCHUNK - 1),
                )
                if a == NCHUNK - 1:
                    nc.vector.tensor_scalar_add(
                        qkT[ct][:, c0:c0 + 512], cell["ps"][:],
                        bqk_sb[:, ct:ct + 1])
            return f
        return [mk(a) for a in range(NCHUNK)]

    def v_unit(t):
        cell = {}

        def mk(a):
            def f():
                if a == 0:
                    cell["ps"] = filp.tile(
                        [P, GQ], f32, name=f"fv{t}", tag="fil")
                nc.tensor.matmul(
                    cell["ps"][:], xT[a][:, t * P:(t + 1) * P], wv_t[:, a, :],
                    start=(a == 0), stop=(a == NCHUNK - 1),
                )
                if a == NCHUNK - 1:
                    nc.vector.tensor_add(
                        vaug[t][:, :, 0:HD],
                        cell["ps"][:].rearrange("p (h d) -> p h d", h=NHL),
                        bvb[:].rearrange("p (h d) -> p h d", h=NHL),
                    )
            return f
        return [mk(a) for a in range(NCHUNK)]

    def proj_unit(mt, w):
        cell = {}

        def mk(a):
            def f():
                if a == 0:
                    cell["ps"] = filp.tile(
                        [P, 512], f32, name=f"fp{mt}_{w}", tag="fil")
                nc.tensor.matmul(
                    cell["ps"][:], wp_t[:, a, mt * P:(mt + 1) * P],
                    oT[a][:, w * 512:(w + 1) * 512],
                    start=(a == 0), stop=(a == 3),
                )
                if a == 3:
                    ys = ysp.tile([P, 512], bf16, name=f"ys{mt}_{w}", tag="ys")
                    nc.vector.tensor_copy(ys[:], cell["ps"][:])
                    nc.sync.dma_start(
                        yT_d[mt * P:(mt + 1) * P, w * 512:(w + 1) * 512],
                        ys[:])
            return f
        return [mk(a) for a in range(4)]

    # pre-phase: transposes interleaved with pair-0 q/k and v units at
    # 512-token granularity, so the PE has work as soon as the first x
    # tiles land instead of idling through the DMA stream.
    def transpose_tg(tg):
        xts = [xtiles[2 * tg], xtiles[2 * tg + 1]]
        tp = scp.tile([P, 2048], bf16, name=f"tp_{tg}", tag="main")
        for r in range(NCHUNK):
            for tt in range(2):
                nc.tensor.transpose(
                    tp[:, r * 256 + tt * P: r * 256 + (tt + 1) * P],
                    xts[tt][:, r * P:(r + 1) * P],
                    identity,
                )
        for r in range(NCHUNK):
            nc.vector.tensor_copy(
                xT[r][:, tg * 256:(tg + 1) * 256],
                tp[:, r * 256:(r + 1) * 256],
            )

    for p in range(4):  # 512-token spans
        transpose_tg(2 * p)
        transpose_tg(2 * p + 1)
        for ct in (0, 4):
            for f in qk_unit(ct, p // 2, p % 2):
                f()
        for t in range(4 * p, 4 * p + 4):
            for f in v_unit(t):
                f()

    # ---------------- attention (software-pipelined) ----------------
    def attn_pair(hp, on_window=None, pre_window=None):
        qt = qkT[hp]
        kt = qkT[4 + hp]
        chunks = []
        for m in range(4):
            for i in range(4 * m + 4):
                chunks.append((m, i))
        pvt = {}   # window -> psum tile
        pts = {}   # idx -> pt tile
        state = {}  # window -> (ev, rcb) awaiting deferred muls
        deferred = []

        def emit_qk(idx):
            m, i = chunks[idx]
            ws = m * 512
            s = max(i * P, ws)
            o = s - ws
            sc = scp.tile([P, 1024], f32, name=f"sc_{hp}_{m}_{i}", tag="main")
            for hh in range(2):
                r0 = hh * HD
                c0 = o if hh == 0 else 512
                nc.tensor.matmul(
                    sc[:, c0:c0 + 512 - o],
                    kt[r0:r0 + HD, i * P:(i + 1) * P],
                    qt[r0:r0 + HD, s:ws + 512],
                    start=True, stop=True,
                )
            pt = ptp.tile([P, 1024], bf16, name=f"pt_{hp}_{m}_{i}", tag="pt")
            nc.scalar.activation(pt[:, o:1024 - o], sc[:, o:1024 - o],
                                 EXP, scale=SCALE)
            if i * P >= ws:  # diagonal chunk: causal mask inside the block
                for hh in range(2):
                    c0 = o if hh == 0 else 512
                    nc.gpsimd.affine_select(
                        out=pt[:, c0:c0 + P],
                        in_=pt[:, c0:c0 + P],
                        compare_op=mybir.AluOpType.is_ge,
                        fill=0.0,
                        base=0,
                        pattern=[[1, P]],
                        channel_multiplier=-1,
                    )
            pts[idx] = pt

        def emit_pv(idx):
            m, i = chunks[idx]
            ws = m * 512
            o = max(i * P, ws) - ws
            if m not in pvt:
                pvt[m] = pvp.tile([P, 1024], f32, name=f"pv_{hp}_{m}",
                                  tag="pv")
            pt = pts.pop(idx)
            for hh in range(2):
                c0 = o if hh == 0 else 512
                nc.tensor.matmul(
                    pvt[m][0:HD + 1, hh * 512 + o:(hh + 1) * 512],
                    vaug[i][:, 2 * hp + hh, :],
                    pt[:, c0:c0 + 512 - o],
                    start=(i == 0), stop=(i == 4 * m + 3),
                )
            if i == 4 * m + 3:
                normalize(m)

        def normalize(m):
            ws = m * 512
            ev = evp.tile([P, 1024], f32, name=f"ev_{hp}_{m}", tag="ev")
            nc.vector.tensor_copy(ev[0:HD + 1, :], pvt[m][0:HD + 1, :])
            del pvt[m]
            # reciprocal of the denominators: reshape [1,1024]->[8,128] via
            # gpsimd-queued DMAs so the DVE recip isn't lane-starved and the
            # sync queue stays out of the chain
            dnT = rcp.tile([8, P], f32, name=f"dnT_{hp}_{m}", tag="dnT")
            nc.gpsimd.dma_start(dnT[:], ev[HD:HD + 1, :])
            rT = rcp.tile([8, P], f32, name=f"rT_{hp}_{m}", tag="rT")
            nc.vector.reciprocal(rT[:], dnT[:])
            rc = rcp.tile([1, 1024], f32, name=f"rc_{hp}_{m}", tag="rc")
            nc.gpsimd.dma_start(rc[:], rT[:])
            rcb = rcp.tile([HD, 1024], f32, name=f"rcb_{hp}_{m}", tag="rcb")
            nc.gpsimd.partition_broadcast(rcb[:], rc[:])
            state[m] = (ev, rcb)

        def muls(m):
            # deferred ~3 chunks after normalize(m): rcb is ready by then, so
            # these never block the DVE FIFO (which recycles filler psum)
            ws = m * 512
            ev, rcb = state.pop(m)
            for hh in range(2):
                nc.vector.tensor_mul(
                    oT[hp][hh * HD:(hh + 1) * HD, ws:ws + 512],
                    ev[0:HD, hh * 512:(hh + 1) * 512],
                    rcb[:, hh * 512:(hh + 1) * 512],
                )
            if on_window is not None:
                on_window(m)

        def emit_pv_s(idx):
            emit_pv(idx)
            m, i = chunks[idx]
            if i == 4 * m + 3:
                deferred.append((idx + 3, lambda m=m: muls(m)))

        for idx in range(len(chunks)):
            while deferred and deferred[0][0] <= idx:
                deferred.pop(0)[1]()
            if chunks[idx][1] == 0 and pre_window is not None:
                pre_window(chunks[idx][0])
            emit_qk(idx)
            if idx >= 1:
                # overstuff the PE so it lags ACT: deps at the FIFO head are
                # then long-resolved -> no micro-stalls -> HAM stays warm
                pump(4 if chunks[idx][1] == 0 else (2 if idx % 3 == 0 else 1), hp)
                emit_pv_s(idx - 1)
        pump(2, hp)
        emit_pv_s(len(chunks) - 1)
        while deferred:
            deferred.pop(0)[1]()

    def chase(m):
        # pair 3 finished window m -> its proj columns are now computable;
        # queue them as fillers for pair 3's remaining windows
        for mt in range(NCHUNK):
            work.extend(proj_unit(mt, m))

    for hp in range(4):
        if hp < 3:
            load_wqk(hp + 1)
            load_wqk(4 + hp + 1)
            if hp == 2:
                wp_t = wpp.tile([P, 4, C], bf16)
                nc.sync.dma_start(wp_t[:], wp_d.rearrange("(a p) n -> p a n", p=P))
            for ct in (hp + 1, 4 + hp + 1):
                for twp in range(2):
                    for sw in range(2):
                        span = 2 * twp + sw
                        if hp == 2 and span >= 1:
                            # defer pair-3 qk for later windows into pair 3
                            # itself (window m reads spans <= m only)
                            for f in qk_unit(ct, twp, sw):
                                vq.append((3, span, f))
                        else:
                            work.extend(qk_unit(ct, twp, sw))
            attn_pair(hp, pre_window=(lambda m, hp=hp: vdrain(hp, m)))
            # qk leftovers for pair hp+1's early windows must complete now
            while work:
                work.pop(0)()
        else:
            attn_pair(hp, on_window=chase,
                      pre_window=(lambda m: vdrain(3, m)))

    # pair 3 done; remaining proj (window 3 queued by chase(3) + leftovers)
    drain()


def _patch_act_tables():
    """Narrow the activation-table chooser so Exp and Ln resolve to the one
    table containing both (act indices/contents on HW are unchanged — this
    only removes lesser alternatives from the insert_act_table_loads pass,
    eliminating per-window table-reload thrash)."""
    import functools

    import concourse.bacc as bacc
    import concourse.hw_specs as hw_specs
    import concourse.mybir as mybir

    orig = hw_specs.get_activation_tables

    @functools.cache
    def patched(module_arch):
        t = {k: set(v) for k, v in orig(module_arch).items()}
        exp, ln = (mybir.ActivationFunctionType.Exp,
                   mybir.ActivationFunctionType.Ln)
        combined = [k for k, v in t.items() if exp in v and ln in v]
        if combined:
            for k, v in t.items():
                if k not in combined:
                    v.discard(exp)
                    v.discard(ln)
        return t

    bacc.get_activation_tables = patched
    return lambda: setattr(bacc, "get_activation_tables", orig)


def _build_program():
    import contextlib

    import concourse.bass as bass
    import concourse.mybir as mybir
    import concourse.tile as tile
    from concourse import bacc

    unpatch = _patch_act_tables()
    nc = bacc.Bacc("TRN2", target_bir_lowering=False, debug=False, num_devices=8)
    f32 = mybir.dt.float32
    bf16 = mybir.dt.bfloat16
    aps = {
        "x": nc.dram_tensor("x", [T, C], bf16, kind="ExternalInput").ap(),
        "wqkv": nc.dram_tensor("wqkv", [C, 3 * GQ], bf16, kind="ExternalInput").ap(),
        "bqk": nc.dram_tensor("bqk", [P, 8], f32, kind="ExternalInput").ap(),
        "bv": nc.dram_tensor("bv", [GQ], f32, kind="ExternalInput").ap(),
        "wp": nc.dram_tensor("wp", [GQ, C], bf16, kind="ExternalInput").ap(),
        "yT": nc.dram_tensor("yT", [C, T], bf16, kind="ExternalOutput").ap(),
    }
    with tile.TileContext(nc) as tc:
        with contextlib.ExitStack() as ctx:
            _emit(ctx, tc, aps, mybir, bass)
    nc.compile()
    unpatch()
    return nc


def get_program():
    global _PROGRAM
    if _PROGRAM is None:
        _PROGRAM = _build_program()
    return _PROGRAM


def make_in_maps(x, w_qkv, b_qkv, w_proj):
    import ml_dtypes

    bf16 = ml_dtypes.bfloat16
    x = np.asarray(x, np.float32)
    w_qkv = np.asarray(w_qkv, np.float32)
    b_qkv = np.asarray(b_qkv, np.float32)
    w_proj = np.asarray(w_proj, np.float32)
    in_maps = []
    for c in range(8):
        b = c // 2
        g = c % 2
        q0 = g * GQ
        wq = w_qkv[:, q0:q0 + GQ]
        wk = w_qkv[:, C + q0:C + q0 + GQ]
        wv = w_qkv[:, 2 * C + q0:2 * C + q0 + GQ]
        wqkv = np.ascontiguousarray(
            np.concatenate([wq, wk, wv], axis=1).astype(bf16))
        bq = b_qkv[q0:q0 + GQ]
        bk = b_qkv[C + q0:C + q0 + GQ]
        bqk = np.ascontiguousarray(np.concatenate([bq, bk]).reshape(8, P).T)
        bv = np.ascontiguousarray(b_qkv[2 * C + q0:2 * C + q0 + GQ])
        in_maps.append({
            "x": np.ascontiguousarray(x[b].astype(bf16)),
            "wqkv": wqkv,
            "bqk": bqk,
            "bv": bv,
            "wp": np.ascontiguousarray(
                w_proj[q0:q0 + GQ, :].astype(bf16)),
        })
    return in_maps


def combine_outputs(outs, b_proj):
    b_proj = np.asarray(b_proj, np.float32)
    y = np.empty((B, T, C), np.float32)
    for b in range(B):
        acc = (outs[2 * b].astype(np.float32)
               + outs[2 * b + 1].astype(np.float32))  # [C, T]
        y[b] = acc.T + b_proj
    return y


def kernel(x, w_qkv, b_qkv, w_proj, b_proj, _trace=False):
    from concourse import bass_utils

    nc = get_program()
    in_maps = make_in_maps(x, w_qkv, b_qkv, w_proj)
    res = bass_utils.run_bass_kernel_spmd(
        nc, in_maps, core_ids=list(range(8)), trace=_trace
    )
    outs = [r["yT"] for r in res.results]
    y = combine_outputs(outs, b_proj)
    if _trace:
        return y, res
    return y
